# revision 30
# baseline (speedup 1.0000x reference)
"""Trainium2 Bass kernel for a pre-LN transformer block (B=4, T=2048, D=1024,
H=16, HS=64, FF=4096, causal attention).

Sharding: data-parallel over batch pairs x 2-way tensor-parallel
(heads for attention, columns/rows for FFN) with a pair AllReduce (bf16)
after the attention output projection and after FC2 (Megatron style).

Core c (0..7): batch b = c//2, TP half = c%2 (8 local heads, 2048 local FF).
Activations are feature-major on chip (d on partitions, t on free dim);
the host transposes x in (bf16) and the output back.

Schedule per 512-token chunk-slot ci (software-pipelined, depth 2-3):
  QKV(ci) -> attention(ci) -> Wo(ci)+AR1(ci) -> LN1(ci+1) -> FC2(ci-1)+AR2(ci-1)
  -> residual2(ci-2)
Attention interleaves the score matmuls of head h with the AV matmuls of
head h-1 so the PE never waits on exp() (which runs one head behind on ACT),
and FC1(ci-1) thunks are injected between score/AV pairs to absorb the
ACT-vs-PE rate gap.  residual1+LN2(ci-1) is emitted two heads into
attention(ci) so its DVE work never blocks and AR1 latency is hidden.
"""

import numpy as np
import ml_dtypes

import concourse.bacc as bacc
import concourse.bass as bass
import concourse.mybir as mybir
import concourse.tile as tile
from concourse.bass_utils import run_bass_kernel_spmd

BF16NP = ml_dtypes.bfloat16

B, T, D, H, HS, FF = 4, 2048, 1024, 16, 64, 4096
EPS = 1e-5
NCORES = 8
TP = 2
LH = H // TP          # 8 local heads
LHE = LH * HS         # 512 local head-embed width
LFF = FF // TP        # 2048 local FF
KD = D // 128         # 8 d k-tiles
KHE = LHE // 128      # 4 he k-tiles
KFF = LFF // 128      # 16 ff k-tiles
NCH = T // 512        # 4 t-chunks of 512
NST = T // 128        # 16 s-tiles of 128
PAIRS = [[0, 1], [2, 3], [4, 5], [6, 7]]

F32 = mybir.dt.float32
BF = mybir.dt.bfloat16


def _emit(nc, tc, t):
    mm = nc.tensor.matmul
    Alu = mybir.AluOpType
    Act = mybir.ActivationFunctionType

    # host pre-shuffles operands so streamed pieces are DRAM-contiguous:
    # xb [NCH,128,KD,512], w1 [8,128,KD,256], w2 [8,128,KFF,128],
    # wq/wk/wv [128,KD,LHE], wo [128,KHE,D]
    xb_t = t["xbT"]
    w1_t = t["w1"]
    w2_t = t["w2"]
    outT_v = t["outT"]

    # ---------------- persistent pools ----------------
    dram = tc.alloc_tile_pool(name="dram", bufs=1, space="DRAM")
    ar1_in = [dram.tile([D, 512], BF, name=f"ar1i{c}") for c in range(NCH)]
    ar1_out = [dram.tile([D, 512], BF, name=f"ar1o{c}") for c in range(NCH)]
    ar2_in = [dram.tile([D, 512], BF, name=f"ar2i{c}") for c in range(NCH)]
    ar2_out = [dram.tile([D, 512], BF, name=f"ar2o{c}") for c in range(NCH)]
    xmid_dram = dram.tile([D, T], BF, name="xmidd")
    xmid_v = xmid_dram.rearrange("(k p) t -> p k t", p=128)

    consts = tc.alloc_tile_pool(name="consts", bufs=1)
    ones_col = consts.tile([128, 1], BF)
    nc.vector.memset(ones_col, 1.0)

    bo_sb = consts.tile([128, KD], F32)
    b2_sb = consts.tile([128, KD], F32)
    b1_sb = consts.tile([128, KFF], F32)
    for name, dst in (("bo", bo_sb), ("b2", b2_sb)):
        nc.sync.dma_start(out=dst, in_=t[name].rearrange("(k p) -> p k", p=128))
    nc.sync.dma_start(out=b1_sb, in_=t["b1l"].rearrange("(k p) -> p k", p=128))

    mask128 = consts.tile([128, 128], BF, name="mask128")
    nc.vector.memset(mask128, 1.0)
    nc.gpsimd.affine_select(
        out=mask128, in_=mask128, compare_op=Alu.is_ge, fill=0.0,
        base=0, channel_multiplier=-1, pattern=[[1, 128]])

    # resident weights: QKV + Wo
    wres = tc.alloc_tile_pool(name="wres", bufs=1)
    wq_sb = wres.tile([128, KD, LHE], BF, tag="wq")
    wk_sb = wres.tile([128, KD, LHE], BF, tag="wk")
    wv_sb = wres.tile([128, KD, LHE], BF, tag="wv")
    wo_sb = wres.tile([128, KHE, D], BF, tag="wo")
    def load_weights():
        for src_, dst in ((t["wq"], wq_sb), (t["wk"], wk_sb),
                          (t["wv"], wv_sb)):
            for s4 in range(4):
                nc.sync.dma_start(out=dst[:, 2 * s4:2 * s4 + 2, :],
                                  in_=src_[:, 2 * s4:2 * s4 + 2, :])
        for s4 in range(4):
            nc.sync.dma_start(out=wo_sb[:, s4, :], in_=t["wo"][:, s4, :])

    # persistent attention state: K and V for the full sequence
    attp = tc.alloc_tile_pool(name="attp", bufs=1)
    kT = attp.tile([128, LH // 2, T], BF, tag="kT")
    vS = attp.tile([128, NST, LH * 65], BF, tag="vS")
    nc.vector.memset(vS, 1.0)

    with tc.tile_pool(name="work", bufs=1) as wp, \
         tc.tile_pool(name="wstream", bufs=1) as wstr, \
         tc.tile_pool(name="scpsum", bufs=2, space="PSUM") as psc, \
         tc.tile_pool(name="oapsum", bufs=2, space="PSUM") as poa, \
         tc.tile_pool(name="genpsum", bufs=2, space="PSUM") as pgen:

        xb_tiles = {}
        hT_tiles = {}
        h2_tiles = {}
        qT_tiles = {}

        def gen_ps(tag="gen", shape=(128, 512)):
            return pgen.tile(list(shape), F32, tag="gen", bufs=2, name=tag)

        # ---------- LN helpers ----------
        def ln_stats(src_k):
            """src_k: list of KD [128,512] bf16 APs. Returns (Ab, Bb) bf16
            broadcast tiles: normalized = src*Ab + Bb."""
            ps_s = gen_ps("ps_s", (1, 512))
            ps_q = gen_ps("ps_q", (1, 512))
            for k in range(KD):
                sq = wp.tile([128, 512], BF, tag="ln_sq", bufs=2, name="ln_sq")
                nc.vector.tensor_mul(out=sq, in0=src_k[k], in1=src_k[k])
                mm(out=ps_s, lhsT=ones_col, rhs=src_k[k],
                   start=(k == 0), stop=(k == KD - 1))
                mm(out=ps_q, lhsT=ones_col, rhs=sq,
                   start=(k == 0), stop=(k == KD - 1))
            return ln_chain(ps_s, ps_q)

        def ln_chain(ps_s, ps_q, pre="ln"):
            m_sb = wp.tile([1, 512], F32, tag=pre + "_m", bufs=1, name="ln_m")
            e2 = wp.tile([1, 512], F32, tag=pre + "_e2", bufs=1, name="ln_e2")
            a_t = wp.tile([1, 512], F32, tag=pre + "_a", bufs=1, name="ln_a")
            nc.vector.tensor_scalar_mul(out=m_sb, in0=ps_s, scalar1=1.0 / D)
            nc.vector.tensor_scalar_mul(out=e2, in0=ps_q, scalar1=1.0 / D)
            nc.vector.tensor_mul(out=a_t, in0=m_sb, in1=m_sb)      # m^2
            nc.vector.scalar_tensor_tensor(                         # var+eps
                out=e2, in0=e2, scalar=EPS, in1=a_t,
                op0=Alu.add, op1=Alu.subtract)
            nc.scalar.activation(out=e2, in_=e2, func=Act.Sqrt)     # sd
            nc.vector.reciprocal(out=a_t, in_=e2)                   # 1/sd
            nc.vector.scalar_tensor_tensor(                         # -m/sd
                out=m_sb, in0=m_sb, scalar=-1.0, in1=a_t,
                op0=Alu.mult, op1=Alu.mult)
            ac = wp.tile([1, 512], BF, tag=pre + "_ac", bufs=1, name="ln_ac")
            bc_ = wp.tile([1, 512], BF, tag=pre + "_bc", bufs=1, name="ln_bc")
            nc.vector.tensor_copy(out=ac, in_=a_t)
            nc.vector.tensor_copy(out=bc_, in_=m_sb)
            Ab = wp.tile([128, 512], BF, tag=pre + "_Ab", bufs=2, name="ln_Ab")
            Bb = wp.tile([128, 512], BF, tag=pre + "_Bb", bufs=2, name="ln_Bb")
            nc.gpsimd.partition_broadcast(Ab, ac)
            nc.gpsimd.partition_broadcast(Bb, bc_)
            return Ab, Bb

        def ln_apply(src, Ab, Bb, out_slice, pre="ln"):
            t1 = wp.tile([128, 512], BF, tag=pre + "_t1", bufs=2,
                         name="ln_t1")
            nc.vector.tensor_mul(out=t1, in0=src, in1=Ab)
            nc.vector.tensor_add(out=out_slice, in0=t1, in1=Bb)

        # ---------- stage functions ----------
        def ln1_load(ci):
            xb = wp.tile([128, KD, 512], BF, tag="xb", bufs=1, name="xb")
            xb_tiles[ci] = xb
            for s4 in range(4):
                nc.sync.dma_start(out=xb[:, 2 * s4:2 * s4 + 2, :],
                                  in_=xb_t[ci][:, 2 * s4:2 * s4 + 2, :])

        def ln1_compute(ci):
            for th in ln1_thunks(ci):
                th()

        def ln1_thunks(ci):
            """LN1(ci) as thunks: per-k stats, the scalar chain, per-k
            apply.  Injectable between attention pairs."""
            xb = xb_tiles[ci]
            hT = wp.tile([128, KD, 512], BF, tag="hT", bufs=1, name="hT")
            hT_tiles[ci] = hT
            box = {}
            thunks = []

            def stats_k(k):
                def f():
                    if k == 0:
                        box["s"] = gen_ps("ps_s", (1, 512))
                        box["q"] = gen_ps("ps_q", (1, 512))
                    sq = wp.tile([128, 512], BF, tag="ln_sq", bufs=2,
                                 name="ln_sq")
                    nc.vector.tensor_mul(out=sq, in0=xb[:, k, :],
                                         in1=xb[:, k, :])
                    mm(out=box["s"], lhsT=ones_col, rhs=xb[:, k, :],
                       start=(k == 0), stop=(k == KD - 1))
                    mm(out=box["q"], lhsT=ones_col, rhs=sq,
                       start=(k == 0), stop=(k == KD - 1))
                return f

            def chain():
                box["AB"] = ln_chain(box["s"], box["q"])

            def apply_k(k):
                def f():
                    Ab, Bb = box["AB"]
                    ln_apply(xb[:, k, :], Ab, Bb, hT[:, k, :])
                return f

            for k in range(KD):
                thunks.append(stats_k(k))
            thunks.append(chain)
            for k in range(KD):
                thunks.append(apply_k(k))
            return thunks

        def qkv(ci):
            c0 = ci * 512
            hT = hT_tiles.pop(ci)
            qT = wp.tile([128, LH // 2, 512], BF, tag="qT", bufs=1, name="qT")
            qT_tiles[ci] = qT
            for et in range(LH // 2):
                ps = gen_ps("ps_proj")
                for k in range(KD):
                    mm(out=ps, lhsT=wk_sb[:, k, et * 128:(et + 1) * 128],
                       rhs=hT[:, k, :],
                       start=(k == 0), stop=(k == KD - 1))
                nc.vector.tensor_copy(out=kT[:, et, c0:c0 + 512], in_=ps)
                ps = gen_ps("ps_proj")
                for k in range(KD):
                    mm(out=ps, lhsT=wq_sb[:, k, et * 128:(et + 1) * 128],
                       rhs=hT[:, k, :],
                       start=(k == 0), stop=(k == KD - 1))
                nc.vector.tensor_copy(out=qT[:, et, :], in_=ps)
            for sti in range(4):
                st = ci * 4 + sti
                ps = gen_ps("ps_v")
                for k in range(KD):
                    mm(out=ps, lhsT=hT[:, k, sti * 128:sti * 128 + 128],
                       rhs=wv_sb[:, k, :],
                       start=(k == 0), stop=(k == KD - 1))
                nc.vector.tensor_copy(
                    out=vS[:, st, :].rearrange("p (h e) -> p h e",
                                               h=LH)[:, :, 0:64],
                    in_=ps.rearrange("p (h e) -> p h e", e=64))

        def res1_ln2(cj):
            for th in res1_ln2_thunks(cj):
                th()

        def res1_ln2_thunks(cj):
            """residual1 + LN2 as injectable thunks: per-k
            [loads, residual STT, store, sq, stats matmuls], scalar chain,
            per-k apply."""
            c0 = cj * 512
            ar1v = ar1_out[cj].rearrange("(k p) t -> p k t", p=128)
            xmid = wp.tile([128, KD, 512], BF, tag="xmid", bufs=1, name="xmid")
            h2 = wp.tile([128, KD, 512], BF, tag="h2", bufs=1, name="h2")
            h2_tiles[cj] = h2
            box = {}
            thunks = []

            def step_k(k):
                def f():
                    if k == 0:
                        box["s"] = gen_ps("ps_s", (1, 512))
                        box["q"] = gen_ps("ps_q", (1, 512))
                    ar_sb = wp.tile([128, 512], BF, tag="arsb", bufs=3,
                                    name="ar1sb")
                    nc.sync.dma_start(out=ar_sb, in_=ar1v[:, k, :])
                    xb2 = wp.tile([128, 512], BF, tag="xb2", bufs=2,
                                  name="xb2")
                    nc.sync.dma_start(out=xb2, in_=xb_t[cj][:, k, :])
                    nc.vector.scalar_tensor_tensor(
                        out=xmid[:, k, :], in0=ar_sb,
                        scalar=bo_sb[:, k:k + 1], in1=xb2,
                        op0=Alu.add, op1=Alu.add)
                    nc.sync.dma_start(out=xmid_v[:, k, c0:c0 + 512],
                                      in_=xmid[:, k, :])
                    sq = wp.tile([128, 512], BF, tag="ln_sq", bufs=2,
                                 name="ln_sq")
                    nc.vector.tensor_mul(out=sq, in0=xmid[:, k, :],
                                         in1=xmid[:, k, :])
                    mm(out=box["s"], lhsT=ones_col, rhs=xmid[:, k, :],
                       start=(k == 0), stop=(k == KD - 1))
                    mm(out=box["q"], lhsT=ones_col, rhs=sq,
                       start=(k == 0), stop=(k == KD - 1))
                return f

            def chain():
                box["AB"] = ln_chain(box["s"], box["q"])

            def apply_k(k):
                def f():
                    Ab2, Bb2 = box["AB"]
                    ln_apply(xmid[:, k, :], Ab2, Bb2, h2[:, k, :])
                return f

            for k in range(KD):
                thunks.append(step_k(k))
            thunks.append(chain)
            for k in range(KD):
                thunks.append(apply_k(k))
            return thunks

        def fc1_thunks(cj):
            """FC1(cj) as a list of ~4-matmul thunks (injected between
            attention pairs).  Call after res1_ln2(cj)."""
            h2 = h2_tiles.pop(cj)
            u = wp.tile([128, KFF, 512], BF, tag="u", bufs=1, name="u")
            u_box[cj] = u
            thunks = []
            w1p_tiles = {}
            ps_box = {}

            def load_w1p(p):
                def f():
                    w1p = wstr.tile([128, KD, 128], BF, tag="w1p", bufs=2,
                                    name="w1p")
                    w1p_tiles[p] = w1p
                    for s4 in range(4):
                        nc.sync.dma_start(out=w1p[:, 2 * s4:2 * s4 + 2, :],
                                          in_=w1_t[p][:, 2 * s4:2 * s4 + 2, :])
                return f

            def up_half(fft, half):
                def f():
                    if half == 0:
                        ps_box[fft] = gen_ps("ps_u")
                    ps = ps_box[fft]
                    w1p = w1p_tiles[fft]
                    for kk in range(4):
                        k = half * 4 + kk
                        mm(out=ps, lhsT=w1p[:, k, :],
                           rhs=h2[:, k, :],
                           start=(k == 0), stop=(k == KD - 1))
                    if half == 1:
                        nc.scalar.activation(out=u[:, fft, :], in_=ps,
                                             func=Act.Relu,
                                             bias=b1_sb[:, fft:fft + 1])
                        del ps_box[fft]
                return f

            load_w1p(0)()
            load_w1p(1)()
            for fft in range(KFF):
                if fft + 2 < KFF:
                    thunks.append(load_w1p(fft + 2))
                thunks.append(up_half(fft, 0))
                thunks.append(up_half(fft, 1))
            return thunks

        def fc2_prefetch(cj):
            """Issue the first two W2 piece DMAs early (right after Wo)."""
            box = {}

            def load_w2p(p):
                w2p = wstr.tile([128, KFF, 128], BF, tag="w2p", bufs=2,
                                name="w2p")
                box[p] = w2p
                for s4 in range(4):
                    nc.sync.dma_start(out=w2p[:, 4 * s4:4 * s4 + 4, :],
                                      in_=w2_t[p][:, 4 * s4:4 * s4 + 4, :])

            box["load"] = load_w2p
            load_w2p(0)
            load_w2p(1)
            w2p_box[cj] = box

        def fc2_ar2(cj, split=False):
            """FC2(cj) as a dense matmul block + bf16 pair AllReduce.
            split=True: two half-AllReduces so the tail can overlap."""
            u = u_box.pop(cj)
            box = w2p_box.pop(cj)
            load_w2p = box["load"]
            halves = ((0, 4), (4, 8)) if split else ((0, 8),)
            for hi, (d0, d1) in enumerate(halves):
                for dt in range(d0, d1):
                    if 2 + dt < KD:
                        load_w2p(2 + dt)
                    ps = gen_ps("ps_f")
                    for k2 in range(KFF):
                        mm(out=ps, lhsT=box[dt][:, k2, :],
                           rhs=u[:, k2, :],
                           start=(k2 == 0), stop=(k2 == KFF - 1))
                    stg = wp.tile([128, 512], BF, tag="stg", bufs=3,
                                  name="stg2")
                    nc.vector.tensor_copy(out=stg, in_=ps)
                    nc.sync.dma_start(
                        out=ar2_in[cj][dt * 128:(dt + 1) * 128, :], in_=stg)
                if split:
                    r0, r1 = d0 * 128, d1 * 128
                    nc.gpsimd.collective_compute(
                        "AllReduce", Alu.add, replica_groups=PAIRS,
                        ins=[ar2_in[cj][r0:r1, :].opt()],
                        outs=[ar2_out[cj][r0:r1, :].opt()])
            if not split:
                nc.gpsimd.collective_compute(
                    "AllReduce", Alu.add, replica_groups=PAIRS,
                    ins=[ar2_in[cj].opt()], outs=[ar2_out[cj].opt()])

        def wo_ar1(ci, oT, split=False):
            halves = ((0, 4), (4, 8)) if split else ((0, 8),)
            for hi, (d0, d1) in enumerate(halves):
                for dt in range(d0, d1):
                    ps = gen_ps("ps_wo")
                    for k in range(KHE):
                        mm(out=ps, lhsT=wo_sb[:, k, dt * 128:(dt + 1) * 128],
                           rhs=oT[:, k, :],
                           start=(k == 0), stop=(k == KHE - 1))
                    stg = wp.tile([128, 512], BF, tag="stg", bufs=3,
                                  name="stg1")
                    nc.vector.tensor_copy(out=stg, in_=ps)
                    nc.sync.dma_start(
                        out=ar1_in[ci][dt * 128:(dt + 1) * 128, :], in_=stg)
                if split:
                    r0, r1 = d0 * 128, d1 * 128
                    nc.gpsimd.collective_compute(
                        "AllReduce", Alu.add, replica_groups=PAIRS,
                        ins=[ar1_in[ci][r0:r1, :].opt()],
                        outs=[ar1_out[ci][r0:r1, :].opt()])
            if not split:
                nc.gpsimd.collective_compute(
                    "AllReduce", Alu.add, replica_groups=PAIRS,
                    ins=[ar1_in[ci].opt()], outs=[ar1_out[ci].opt()])

        def res2(cj, ks=range(KD)):
            c0 = cj * 512
            ar2v = ar2_out[cj].rearrange("(k p) t -> p k t", p=128)
            for k in ks:
                a2 = wp.tile([128, 512], BF, tag="arsb", bufs=3, name="ar2sb")
                nc.sync.dma_start(out=a2, in_=ar2v[:, k, :])
                xm = wp.tile([128, 512], BF, tag="xm2", bufs=2, name="xm2")
                nc.sync.dma_start(out=xm, in_=xmid_v[:, k, c0:c0 + 512])
                o_f = wp.tile([128, 512], F32, tag="o_f", bufs=2, name="o_f")
                nc.vector.scalar_tensor_tensor(
                    out=o_f, in0=a2, scalar=b2_sb[:, k:k + 1], in1=xm,
                    op0=Alu.add, op1=Alu.add)
                nc.sync.dma_start(
                    out=outT_v[k * 128:(k + 1) * 128, c0:c0 + 512], in_=o_f)

        u_box = {}
        w2p_box = {}

        # ---------- attention for one chunk (head-interleaved) ----------
        def att_block(ci, res1_head=1):
            nb = 4 * (ci + 1)
            ngrp = nb // 2
            qT = qT_tiles.pop(ci)
            dn8 = wp.tile([LH, 512], F32, tag="dn8", bufs=2, name="dn8")
            oT = wp.tile([128, KHE, 512], BF, tag="oT", bufs=1, name="oT")
            ou_map = {}
            ex_map = {}
            po_map = {}
            pending = []
            pending_hi = []
            state = {"i": 0, "hi": 0, "per": 0, "reserve": 0}

            def inject(ignore_reserve=False):
                lim = len(pending) - (0 if ignore_reserve else
                                      state["reserve"])
                for _ in range(state["per"]):
                    if state["hi"] < len(pending_hi):
                        pending_hi[state["hi"]]()
                        state["hi"] += 1
                    elif state["i"] < lim:
                        pending[state["i"]]()
                        state["i"] += 1

            def repace(h):
                # spread remaining thunks over remaining pair slots,
                # keeping `reserve` of them for the normalization window
                slots = (LH - 1 - h) * ngrp + ngrp
                rem = (len(pending) - state["i"] - state["reserve"]
                       + len(pending_hi) - state["hi"])
                state["per"] = max(1, -(-rem // max(slots, 1)))

            def emit_S(h, g):
                hp, hi = h // 2, h % 2
                e0 = hi * 64
                grp = psc.tile([128, 2, 512], F32, tag="ps_sc", bufs=2,
                               name="ps_sc")
                for j in range(2):
                    sb = 2 * g + j
                    mm(out=grp[:, j, :],
                       lhsT=kT[e0:e0 + 64, hp, sb * 128:(sb + 1) * 128],
                       rhs=qT[e0:e0 + 64, hp, :],
                       start=True, stop=True)
                exg = wp.tile([128, 2, 512], BF, tag="ex", bufs=10, name="ex")
                ex_map[(h, g)] = exg
                base = 4 * ci
                cuts = [max(0, (2 * g + j - base)) * 128
                        if 2 * g + j >= base else 0 for j in range(2)]
                if cuts == [0, 0]:
                    nc.scalar.activation(out=exg, in_=grp, func=Act.Exp,
                                         scale=float(HS) ** -0.5)
                else:
                    # causal: columns below the diagonal offset are dead --
                    # exp/mask/AV only touch the live range
                    for j in range(2):
                        cut = cuts[j]
                        nc.scalar.activation(out=exg[:, j, cut:512],
                                             in_=grp[:, j, cut:512],
                                             func=Act.Exp,
                                             scale=float(HS) ** -0.5)
                for j in range(2):
                    sb = 2 * g + j
                    if sb >= base:
                        cut = cuts[j]
                        nc.vector.tensor_mul(
                            out=exg[:, j, cut:cut + 128],
                            in0=exg[:, j, cut:cut + 128], in1=mask128)

            def emit_AV(h, g):
                if g == 0:
                    po_map[h] = poa.tile([65, 512], F32, tag="po", bufs=2,
                                         name="po")
                po = po_map[h]
                exg = ex_map.pop((h, g))
                base = 4 * ci
                for j in range(2):
                    sb = 2 * g + j
                    cut = (max(0, (sb - base)) * 128 if sb >= base else 0)
                    mm(out=po[:, cut:512],
                       lhsT=vS[:, sb, h * 65:h * 65 + 65],
                       rhs=exg[:, j, cut:512],
                       start=(sb == 0), stop=(sb == nb - 1))
                if g == ngrp - 1:
                    ou = wp.tile([64, 512], BF, tag="ou", bufs=9, name="ou")
                    ou_map[h] = ou
                    nc.vector.tensor_copy(out=ou, in_=po[0:64, :])
                    dnr = wp.tile([1, 512], F32, tag="dnr", bufs=2,
                                  name="dnr")
                    nc.vector.tensor_copy(out=dnr, in_=po[64:65, :])
                    nc.sync.dma_start(out=dn8[h:h + 1, :], in_=dnr)

            for h in range(LH):
                for g in range(ngrp):
                    emit_S(h, g)
                    if h > 0:
                        emit_AV(h - 1, g)
                    inject()
                if h == res1_head and ci >= 1:
                    res1_ln2(ci - 1)
                    pending.extend(fc1_thunks(ci - 1))
                    state["reserve"] = 10
                    repace(h)
                if h == 1 and ci == 0:
                    pending_hi.extend(ln1_thunks(1))
                    repace(h)
            for g in range(ngrp):
                emit_AV(LH - 1, g)
                inject()

            # normalization: o /= rowsum(exp); remaining thunks keep PE busy
            rec8 = wp.tile([LH, 512], F32, tag="rec8", bufs=1, name="rec8")
            nc.vector.reciprocal(out=rec8, in_=dn8)
            rb8 = wp.tile([LH, 512], BF, tag="rb8", bufs=1, name="rb8")
            nc.vector.tensor_copy(out=rb8, in_=rec8)
            for h in range(LH):
                rbt = wp.tile([1, 512], BF, tag="rbt", bufs=2, name="rbt")
                nc.sync.dma_start(out=rbt, in_=rb8[h:h + 1, :])
                bc = wp.tile([64, 512], BF, tag="bc", bufs=2, name="bc")
                nc.gpsimd.partition_broadcast(bc, rbt)
                nc.vector.tensor_mul(
                    out=oT[(h % 2) * 64:(h % 2) * 64 + 64, h // 2, :],
                    in0=ou_map[h], in1=bc)
                state["per"] = 2
                inject(ignore_reserve=True)
            state["per"] = len(pending) + len(pending_hi)
            inject(ignore_reserve=True)
            return oT

        # ---------- main schedule ----------
        ln1_load(0)
        load_weights()
        ln1_compute(0)
        for ci in range(NCH):
            qkv(ci)
            if ci + 1 < NCH:
                ln1_load(ci + 1)
            oT = att_block(ci, res1_head=(2 if ci == 1 else 1))
            last = ci == NCH - 1
            wo_ar1(ci, oT, split=last)
            if ci >= 1:
                fc2_prefetch(ci - 1)
            if 1 <= ci < NCH - 1:
                ln1_compute(ci + 1)
            if ci >= 1:
                fc2_ar2(ci - 1)
            if ci >= 2:
                res2(ci - 2)
        # ---------- tail ----------
        res1_ln2(NCH - 1)
        fc2_prefetch(NCH - 1)
        for th in fc1_thunks(NCH - 1):
            th()
        res2(NCH - 2)
        fc2_ar2(NCH - 1, split=True)
        res2(NCH - 1, ks=range(0, 4))
        res2(NCH - 1, ks=range(4, KD))

    attp.release()
    wres.release()
    consts.release()
    dram.release()


def _build():
    nc = bacc.Bacc("TRN2", target_bir_lowering=False, debug=False,
                   num_devices=NCORES)

    tensors = {}
    tensors["xbT"] = nc.dram_tensor("xbT", [NCH, 128, KD, 512], BF,
                                    kind="ExternalInput").ap()
    for name, shape, dt in (
        ("wq", [128, KD, LHE], BF), ("wk", [128, KD, LHE], BF),
        ("wv", [128, KD, LHE], BF),
        ("wo", [128, KHE, D], BF), ("w1", [KFF, 128, KD, 128], BF),
        ("w2", [KD, 128, KFF, 128], BF),
        ("b1l", [LFF], F32), ("bo", [D], F32), ("b2", [D], F32),
    ):
        tensors[name] = nc.dram_tensor(name, shape, dt,
                                       kind="ExternalInput").ap()
    tensors["outT"] = nc.dram_tensor("out", [D, T], F32,
                                     kind="ExternalOutput").ap()

    with tile.TileContext(nc, num_cores=NCORES) as tc:
        _emit(nc, tc, tensors)

    nc.compile()
    return nc


_NC_CACHE = None


def _get_nc():
    global _NC_CACHE
    if _NC_CACHE is None:
        _NC_CACHE = _build()
    return _NC_CACHE


def _shard_inputs(x, Wq, Wk, Wv, Wo, bo, W1, b1, W2, b2, g1, be1, g2, be2):
    """Build the 8 per-core input maps."""
    bf = lambda a: np.ascontiguousarray(a).astype(BF16NP)
    f32 = lambda a: np.ascontiguousarray(a, dtype=np.float32)

    in_maps = []
    for c in range(NCORES):
        b, half = divmod(c, TP)
        heads = slice(half * LH, (half + 1) * LH)
        ffs = slice(half * LFF, (half + 1) * LFF)
        hes = slice(half * LHE, (half + 1) * LHE)
        g1a = np.asarray(g1, dtype=np.float64)
        be1a = np.asarray(be1, dtype=np.float64)
        g2a = np.asarray(g2, dtype=np.float64)
        be2a = np.asarray(be2, dtype=np.float64)
        wq_l = np.concatenate(list(np.asarray(Wq)[heads]), axis=1)
        wk_l = np.concatenate(list(np.asarray(Wk)[heads]), axis=1)
        wv_l = np.concatenate(list(np.asarray(Wv)[heads]), axis=1)
        # fold LN1 gamma into Wq/Wk/Wv; q keeps a bias, k's bias cancels in
        # softmax, v's bias folds into bo (summed over ALL heads, not just
        # this core's, since both TP halves add bo after the AllReduce)
        # q bias be1@Wq is exactly zero for this model (be1 == 0)
        assert not np.any(be1a), "nonzero be1 needs a q-bias add"
        wq_l = wq_l * g1a[:, None]
        wk_l = wk_l * g1a[:, None]
        wv_all = np.concatenate(list(np.asarray(Wv)), axis=1)
        bo_f = (np.asarray(bo, dtype=np.float64)
                + (be1a @ wv_all) @ np.asarray(Wo, dtype=np.float64))
        wv_l = wv_l * g1a[:, None]
        # fold LN2 gamma/beta into W1/b1
        w1_loc = np.asarray(W1)[:, ffs]
        b1_f = np.asarray(b1, dtype=np.float64)[ffs] + be2a @ w1_loc
        w1_loc = w1_loc * g2a[:, None]
        # pre-shuffle so every on-chip tile/piece is DRAM-contiguous
        shuf_kp = lambda w, wid: np.ascontiguousarray(
            w.reshape(KD, 128, wid).transpose(1, 0, 2))      # [128, KD, wid]
        w1_l = w1_loc.reshape(KD, 128, KFF, 128)
        w1_l = np.ascontiguousarray(w1_l.transpose(2, 1, 0, 3))
        w2_l = np.asarray(W2)[ffs, :].reshape(KFF, 128, KD, 128)
        w2_l = np.ascontiguousarray(w2_l.transpose(2, 1, 0, 3))
        xb_l = np.asarray(x)[b].T.reshape(KD, 128, NCH, 512)
        xb_l = np.ascontiguousarray(xb_l.transpose(2, 1, 0, 3))
        wo_l = np.asarray(Wo)[hes, :].reshape(KHE, 128, D)
        wo_l = np.ascontiguousarray(wo_l.transpose(1, 0, 2))
        in_maps.append({
            "xbT": bf(xb_l),
            "wq": bf(shuf_kp(wq_l, LHE)), "wk": bf(shuf_kp(wk_l, LHE)),
            "wv": bf(shuf_kp(wv_l, LHE)),
            "wo": bf(wo_l),
            "w1": bf(w1_l), "w2": bf(w2_l),
            "b1l": f32(b1_f),
            "bo": f32(bo_f), "b2": f32(b2),
        })
    return in_maps


def kernel(x, Wq, Wk, Wv, Wo, bo, W1, b1, W2, b2, g1, be1, g2, be2,
           _trace=False):
    nc = _get_nc()
    in_maps = _shard_inputs(x, Wq, Wk, Wv, Wo, bo, W1, b1, W2, b2,
                            g1, be1, g2, be2)
    res = run_bass_kernel_spmd(nc, in_maps, list(range(NCORES)),
                               trace=_trace)
    out = np.empty((B, T, D), dtype=np.float32)
    for b in range(B):
        out[b] = res.results[TP * b]["out"].T
    if _trace:
        kernel.last_exec_time_ns = res.exec_time_ns
        kernel.last_results = res
    return out


# revision 32
# speedup vs baseline: 1.0992x; 1.0992x over previous
"""Trainium2 Bass kernel for a pre-LN transformer block (B=4, T=2048, D=1024,
H=16, HS=64, FF=4096, causal attention).

Sharding: data-parallel over batch pairs x 2-way tensor-parallel
(heads for attention, columns/rows for FFN) with a pair AllReduce (bf16)
after the attention output projection and after FC2 (Megatron style).

Core c (0..7): batch b = c//2, TP half = c%2 (8 local heads, 2048 local FF).
Activations are feature-major on chip (d on partitions, t on free dim);
the host transposes x in (bf16) and the output back.

Schedule per 512-token chunk-slot ci (software-pipelined, depth 2-3):
  QKV(ci) -> attention(ci) -> Wo(ci)+AR1(ci) -> LN1(ci+1) -> FC2(ci-1)+AR2(ci-1)
  -> residual2(ci-2)
Attention interleaves the score matmuls of head h with the AV matmuls of
head h-1 so the PE never waits on exp() (which runs one head behind on ACT),
and FC1(ci-1) thunks are injected between score/AV pairs to absorb the
ACT-vs-PE rate gap.  residual1+LN2(ci-1) is emitted two heads into
attention(ci) so its DVE work never blocks and AR1 latency is hidden.
"""

import numpy as np
import ml_dtypes

import concourse.bacc as bacc
import concourse.bass as bass
import concourse.mybir as mybir
import concourse.tile as tile
from concourse.bass_utils import run_bass_kernel_spmd

BF16NP = ml_dtypes.bfloat16

B, T, D, H, HS, FF = 4, 2048, 1024, 16, 64, 4096
EPS = 1e-5
NCORES = 8
TP = 2
LH = H // TP          # 8 local heads
LHE = LH * HS         # 512 local head-embed width
LFF = FF // TP        # 2048 local FF
KD = D // 128         # 8 d k-tiles
KHE = LHE // 128      # 4 he k-tiles
KFF = LFF // 128      # 16 ff k-tiles
NCH = T // 512        # 4 t-chunks of 512
NST = T // 128        # 16 s-tiles of 128
PAIRS = [[0, 1], [2, 3], [4, 5], [6, 7]]

F32 = mybir.dt.float32
BF = mybir.dt.bfloat16


def _emit(nc, tc, t):
    mm = nc.tensor.matmul
    Alu = mybir.AluOpType
    Act = mybir.ActivationFunctionType

    # host pre-shuffles operands so streamed pieces are DRAM-contiguous:
    # xb [NCH,128,KD,512], w1 [8,128,KD,256], w2 [8,128,KFF,128],
    # wq/wk/wv [128,KD,LHE], wo [128,KHE,D]
    xb_t = t["xbT"]
    w1_t = t["w1"]
    w2_t = t["w2"]
    outT_v = t["outT"]

    # ---------------- persistent pools ----------------
    dram = tc.alloc_tile_pool(name="dram", bufs=1, space="DRAM")
    ar1_in = [dram.tile([D, 512], BF, name=f"ar1i{c}") for c in range(NCH)]
    ar1_out = [dram.tile([D, 512], BF, name=f"ar1o{c}") for c in range(NCH)]
    ar2_in = [dram.tile([D, 512], BF, name=f"ar2i{c}") for c in range(NCH)]
    ar2_out = [dram.tile([D, 512], BF, name=f"ar2o{c}") for c in range(NCH)]
    xmid_dram = dram.tile([D, T], BF, name="xmidd")
    xmid_v = xmid_dram.rearrange("(k p) t -> p k t", p=128)

    consts = tc.alloc_tile_pool(name="consts", bufs=1)
    ones_col = consts.tile([128, 1], BF)
    nc.vector.memset(ones_col, 1.0)

    bo_sb = consts.tile([128, KD], F32)
    b2_sb = consts.tile([128, KD], F32)
    b1_sb = consts.tile([128, KFF], F32)
    for name, dst in (("bo", bo_sb), ("b2", b2_sb)):
        nc.sync.dma_start(out=dst, in_=t[name].rearrange("(k p) -> p k", p=128))
    nc.sync.dma_start(out=b1_sb, in_=t["b1l"].rearrange("(k p) -> p k", p=128))

    mask128 = consts.tile([128, 128], BF, name="mask128")
    nc.vector.memset(mask128, 1.0)
    nc.gpsimd.affine_select(
        out=mask128, in_=mask128, compare_op=Alu.is_ge, fill=0.0,
        base=0, channel_multiplier=-1, pattern=[[1, 128]])

    # resident weights: QKV + Wo
    wres = tc.alloc_tile_pool(name="wres", bufs=1)
    wq_sb = wres.tile([128, KD, LHE], BF, tag="wq")
    wk_sb = wres.tile([128, KD, LHE], BF, tag="wk")
    wv_sb = wres.tile([128, KD, LHE], BF, tag="wv")
    wo_sb = wres.tile([128, KHE, D], BF, tag="wo")
    def load_weights():
        for src_, dst in ((t["wq"], wq_sb), (t["wk"], wk_sb),
                          (t["wv"], wv_sb)):
            for s4 in range(4):
                nc.sync.dma_start(out=dst[:, 2 * s4:2 * s4 + 2, :],
                                  in_=src_[:, 2 * s4:2 * s4 + 2, :])
        for s4 in range(4):
            nc.sync.dma_start(out=wo_sb[:, s4, :], in_=t["wo"][:, s4, :])

    # persistent attention state: K and V for the full sequence
    attp = tc.alloc_tile_pool(name="attp", bufs=1)
    kT = attp.tile([128, LH // 2, T], BF, tag="kT")
    vS = attp.tile([128, NST, LH * 65], BF, tag="vS")
    nc.vector.memset(vS, 1.0)

    with tc.tile_pool(name="work", bufs=1) as wp, \
         tc.tile_pool(name="wstream", bufs=1) as wstr, \
         tc.tile_pool(name="scpsum", bufs=2, space="PSUM") as psc, \
         tc.tile_pool(name="oapsum", bufs=2, space="PSUM") as poa, \
         tc.tile_pool(name="genpsum", bufs=2, space="PSUM") as pgen:

        xb_tiles = {}
        hT_tiles = {}
        h2_tiles = {}
        qT_tiles = {}

        def gen_ps(tag="gen", shape=(128, 512)):
            return pgen.tile(list(shape), F32, tag="gen", bufs=2, name=tag)

        # ---------- LN helpers ----------
        def ln_stats(src_k):
            """src_k: list of KD [128,512] bf16 APs. Returns (Ab, Bb) bf16
            broadcast tiles: normalized = src*Ab + Bb."""
            ps_s = gen_ps("ps_s", (1, 512))
            ps_q = gen_ps("ps_q", (1, 512))
            for k in range(KD):
                sq = wp.tile([128, 512], BF, tag="ln_sq", bufs=2, name="ln_sq")
                nc.vector.tensor_mul(out=sq, in0=src_k[k], in1=src_k[k])
                mm(out=ps_s, lhsT=ones_col, rhs=src_k[k],
                   start=(k == 0), stop=(k == KD - 1))
                mm(out=ps_q, lhsT=ones_col, rhs=sq,
                   start=(k == 0), stop=(k == KD - 1))
            return ln_chain(ps_s, ps_q)

        def ln_chain(ps_s, ps_q, pre="ln"):
            m_sb = wp.tile([1, 512], F32, tag=pre + "_m", bufs=1, name="ln_m")
            e2 = wp.tile([1, 512], F32, tag=pre + "_e2", bufs=1, name="ln_e2")
            a_t = wp.tile([1, 512], F32, tag=pre + "_a", bufs=1, name="ln_a")
            nc.vector.tensor_scalar_mul(out=m_sb, in0=ps_s, scalar1=1.0 / D)
            nc.vector.tensor_scalar_mul(out=e2, in0=ps_q, scalar1=1.0 / D)
            nc.vector.tensor_mul(out=a_t, in0=m_sb, in1=m_sb)      # m^2
            nc.vector.scalar_tensor_tensor(                         # var+eps
                out=e2, in0=e2, scalar=EPS, in1=a_t,
                op0=Alu.add, op1=Alu.subtract)
            nc.scalar.activation(out=e2, in_=e2, func=Act.Sqrt)     # sd
            nc.vector.reciprocal(out=a_t, in_=e2)                   # 1/sd
            nc.vector.scalar_tensor_tensor(                         # -m/sd
                out=m_sb, in0=m_sb, scalar=-1.0, in1=a_t,
                op0=Alu.mult, op1=Alu.mult)
            ac = wp.tile([1, 512], BF, tag=pre + "_ac", bufs=1, name="ln_ac")
            bc_ = wp.tile([1, 512], BF, tag=pre + "_bc", bufs=1, name="ln_bc")
            nc.vector.tensor_copy(out=ac, in_=a_t)
            nc.vector.tensor_copy(out=bc_, in_=m_sb)
            Ab = wp.tile([128, 512], BF, tag=pre + "_Ab", bufs=2, name="ln_Ab")
            Bb = wp.tile([128, 512], BF, tag=pre + "_Bb", bufs=2, name="ln_Bb")
            nc.gpsimd.partition_broadcast(Ab, ac)
            nc.gpsimd.partition_broadcast(Bb, bc_)
            return Ab, Bb

        def ln_apply(src, Ab, Bb, out_slice, pre="ln"):
            t1 = wp.tile([128, 512], BF, tag=pre + "_t1", bufs=2,
                         name="ln_t1")
            nc.vector.tensor_mul(out=t1, in0=src, in1=Ab)
            nc.vector.tensor_add(out=out_slice, in0=t1, in1=Bb)

        # ---------- stage functions ----------
        def ln1_load(ci):
            xb = wp.tile([128, KD, 512], BF, tag="xb", bufs=1, name="xb")
            xb_tiles[ci] = xb
            for s8 in range(8):
                nc.sync.dma_start(out=xb[:, s8:s8 + 1, :],
                                  in_=xb_t[ci][:, s8:s8 + 1, :])

        def ln1_compute(ci):
            for th in ln1_thunks(ci):
                th()

        def ln1_thunks(ci):
            """LN1(ci) as thunks: per-k stats, the scalar chain, per-k
            apply.  Injectable between attention pairs."""
            xb = xb_tiles[ci]
            hT = wp.tile([128, KD, 512], BF, tag="hT", bufs=1, name="hT")
            hT_tiles[ci] = hT
            box = {}
            thunks = []

            def stats_k(k):
                def f():
                    if k == 0:
                        box["s"] = gen_ps("ps_s", (1, 512))
                        box["q"] = gen_ps("ps_q", (1, 512))
                    sq = wp.tile([128, 512], BF, tag="ln_sq", bufs=2,
                                 name="ln_sq")
                    nc.vector.tensor_mul(out=sq, in0=xb[:, k, :],
                                         in1=xb[:, k, :])
                    mm(out=box["s"], lhsT=ones_col, rhs=xb[:, k, :],
                       start=(k == 0), stop=(k == KD - 1))
                    mm(out=box["q"], lhsT=ones_col, rhs=sq,
                       start=(k == 0), stop=(k == KD - 1))
                return f

            def chain():
                box["AB"] = ln_chain(box["s"], box["q"])

            def apply_k(k):
                def f():
                    Ab, Bb = box["AB"]
                    ln_apply(xb[:, k, :], Ab, Bb, hT[:, k, :])
                return f

            for k in range(KD):
                thunks.append(stats_k(k))
            thunks.append(chain)
            for k in range(KD):
                thunks.append(apply_k(k))
            return thunks

        def qkv(ci):
            c0 = ci * 512
            hT = hT_tiles.pop(ci)
            qT = wp.tile([128, LH // 2, 512], BF, tag="qT", bufs=1, name="qT")
            qT_tiles[ci] = qT
            for et in range(LH // 2):
                ps = gen_ps("ps_proj")
                for k in range(KD):
                    mm(out=ps, lhsT=wk_sb[:, k, et * 128:(et + 1) * 128],
                       rhs=hT[:, k, :],
                       start=(k == 0), stop=(k == KD - 1))
                nc.vector.tensor_copy(out=kT[:, et, c0:c0 + 512], in_=ps)
                ps = gen_ps("ps_proj")
                for k in range(KD):
                    mm(out=ps, lhsT=wq_sb[:, k, et * 128:(et + 1) * 128],
                       rhs=hT[:, k, :],
                       start=(k == 0), stop=(k == KD - 1))
                nc.vector.tensor_copy(out=qT[:, et, :], in_=ps)
            for sti in range(4):
                st = ci * 4 + sti
                ps = gen_ps("ps_v")
                for k in range(KD):
                    mm(out=ps, lhsT=hT[:, k, sti * 128:sti * 128 + 128],
                       rhs=wv_sb[:, k, :],
                       start=(k == 0), stop=(k == KD - 1))
                nc.vector.tensor_copy(
                    out=vS[:, st, :].rearrange("p (h e) -> p h e",
                                               h=LH)[:, :, 0:64],
                    in_=ps.rearrange("p (h e) -> p h e", e=64))

        def res1_ln2(cj):
            for th in res1_ln2_thunks(cj):
                th()

        def res1_ln2_thunks(cj):
            """residual1 + LN2 as injectable thunks: per-k
            [loads, residual STT, store, sq, stats matmuls], scalar chain,
            per-k apply."""
            c0 = cj * 512
            ar1v = ar1_out[cj].rearrange("(k p) t -> p k t", p=128)
            xmid = wp.tile([128, KD, 512], BF, tag="xmid", bufs=1, name="xmid")
            h2 = wp.tile([128, KD, 512], BF, tag="h2", bufs=1, name="h2")
            h2_tiles[cj] = h2
            box = {}
            thunks = []

            def step_k(k):
                def f():
                    if k == 0:
                        box["s"] = gen_ps("ps_s", (1, 512))
                        box["q"] = gen_ps("ps_q", (1, 512))
                    ar_sb = wp.tile([128, 512], BF, tag="arsb", bufs=3,
                                    name="ar1sb")
                    nc.sync.dma_start(out=ar_sb, in_=ar1v[:, k, :])
                    xb2 = wp.tile([128, 512], BF, tag="xb2", bufs=2,
                                  name="xb2")
                    nc.sync.dma_start(out=xb2, in_=xb_t[cj][:, k, :])
                    nc.vector.scalar_tensor_tensor(
                        out=xmid[:, k, :], in0=ar_sb,
                        scalar=bo_sb[:, k:k + 1], in1=xb2,
                        op0=Alu.add, op1=Alu.add)
                    nc.sync.dma_start(out=xmid_v[:, k, c0:c0 + 512],
                                      in_=xmid[:, k, :])
                    sq = wp.tile([128, 512], BF, tag="ln_sq", bufs=2,
                                 name="ln_sq")
                    nc.vector.tensor_mul(out=sq, in0=xmid[:, k, :],
                                         in1=xmid[:, k, :])
                    mm(out=box["s"], lhsT=ones_col, rhs=xmid[:, k, :],
                       start=(k == 0), stop=(k == KD - 1))
                    mm(out=box["q"], lhsT=ones_col, rhs=sq,
                       start=(k == 0), stop=(k == KD - 1))
                return f

            def chain():
                box["AB"] = ln_chain(box["s"], box["q"])

            def apply_k(k):
                def f():
                    Ab2, Bb2 = box["AB"]
                    ln_apply(xmid[:, k, :], Ab2, Bb2, h2[:, k, :])
                return f

            for k in range(KD):
                thunks.append(step_k(k))
            thunks.append(chain)
            for k in range(KD):
                thunks.append(apply_k(k))
            return thunks

        def fc1_thunks(cj):
            """FC1(cj) as a list of ~4-matmul thunks (injected between
            attention pairs).  Call after res1_ln2(cj)."""
            h2 = h2_tiles.pop(cj)
            u = wp.tile([128, KFF, 512], BF, tag="u", bufs=1, name="u")
            u_box[cj] = u
            thunks = []
            w1p_tiles = {}
            ps_box = {}

            def load_w1p(p):
                def f():
                    w1p = wstr.tile([128, KD, 128], BF, tag="w1p", bufs=2,
                                    name="w1p")
                    w1p_tiles[p] = w1p
                    for s4 in range(4):
                        nc.sync.dma_start(out=w1p[:, 2 * s4:2 * s4 + 2, :],
                                          in_=w1_t[p][:, 2 * s4:2 * s4 + 2, :])
                return f

            def up_half(fft, half):
                def f():
                    if half == 0:
                        ps_box[fft] = gen_ps("ps_u")
                    ps = ps_box[fft]
                    w1p = w1p_tiles[fft]
                    for kk in range(4):
                        k = half * 4 + kk
                        mm(out=ps, lhsT=w1p[:, k, :],
                           rhs=h2[:, k, :],
                           start=(k == 0), stop=(k == KD - 1))
                    if half == 1:
                        nc.vector.tensor_scalar(
                            out=u[:, fft, :], in0=ps,
                            scalar1=b1_sb[:, fft:fft + 1], scalar2=0.0,
                            op0=Alu.add, op1=Alu.max)
                        del ps_box[fft]
                return f

            load_w1p(0)()
            load_w1p(1)()
            for fft in range(KFF):
                if fft + 2 < KFF:
                    thunks.append(load_w1p(fft + 2))
                thunks.append(up_half(fft, 0))
                thunks.append(up_half(fft, 1))
            return thunks

        def fc2_prefetch(cj):
            """Issue the first two W2 piece DMAs early (right after Wo)."""
            box = {}

            def load_w2p(p):
                w2p = wstr.tile([128, KFF, 128], BF, tag="w2p", bufs=2,
                                name="w2p")
                box[p] = w2p
                for s4 in range(4):
                    nc.sync.dma_start(out=w2p[:, 4 * s4:4 * s4 + 4, :],
                                      in_=w2_t[p][:, 4 * s4:4 * s4 + 4, :])

            box["load"] = load_w2p
            load_w2p(0)
            load_w2p(1)
            w2p_box[cj] = box

        def fc2_ar2(cj, split=False):
            """FC2(cj) as a dense matmul block + bf16 pair AllReduce.
            split=True: two half-AllReduces so the tail can overlap."""
            u = u_box.pop(cj)
            box = w2p_box.pop(cj)
            load_w2p = box["load"]
            halves = ((0, 4), (4, 8)) if split else ((0, 8),)
            for hi, (d0, d1) in enumerate(halves):
                for dt in range(d0, d1):
                    if 2 + dt < KD:
                        load_w2p(2 + dt)
                    ps = gen_ps("ps_f")
                    for k2 in range(KFF):
                        mm(out=ps, lhsT=box[dt][:, k2, :],
                           rhs=u[:, k2, :],
                           start=(k2 == 0), stop=(k2 == KFF - 1))
                    stg = wp.tile([128, 512], BF, tag="stg", bufs=3,
                                  name="stg2")
                    nc.vector.tensor_copy(out=stg, in_=ps)
                    nc.sync.dma_start(
                        out=ar2_in[cj][dt * 128:(dt + 1) * 128, :], in_=stg)
                if split:
                    r0, r1 = d0 * 128, d1 * 128
                    nc.gpsimd.collective_compute(
                        "AllReduce", Alu.add, replica_groups=PAIRS,
                        ins=[ar2_in[cj][r0:r1, :].opt()],
                        outs=[ar2_out[cj][r0:r1, :].opt()])
            if not split:
                nc.gpsimd.collective_compute(
                    "AllReduce", Alu.add, replica_groups=PAIRS,
                    ins=[ar2_in[cj].opt()], outs=[ar2_out[cj].opt()])

        def wo_ar1(ci, oT, split=False):
            halves = ((0, 4), (4, 8)) if split else ((0, 8),)
            for hi, (d0, d1) in enumerate(halves):
                for dt in range(d0, d1):
                    ps = gen_ps("ps_wo")
                    for k in range(KHE):
                        mm(out=ps, lhsT=wo_sb[:, k, dt * 128:(dt + 1) * 128],
                           rhs=oT[:, k, :],
                           start=(k == 0), stop=(k == KHE - 1))
                    stg = wp.tile([128, 512], BF, tag="stg", bufs=3,
                                  name="stg1")
                    nc.vector.tensor_copy(out=stg, in_=ps)
                    nc.sync.dma_start(
                        out=ar1_in[ci][dt * 128:(dt + 1) * 128, :], in_=stg)
                if split:
                    r0, r1 = d0 * 128, d1 * 128
                    nc.gpsimd.collective_compute(
                        "AllReduce", Alu.add, replica_groups=PAIRS,
                        ins=[ar1_in[ci][r0:r1, :].opt()],
                        outs=[ar1_out[ci][r0:r1, :].opt()])
            if not split:
                nc.gpsimd.collective_compute(
                    "AllReduce", Alu.add, replica_groups=PAIRS,
                    ins=[ar1_in[ci].opt()], outs=[ar1_out[ci].opt()])

        def res2(cj, ks=range(KD)):
            c0 = cj * 512
            ar2v = ar2_out[cj].rearrange("(k p) t -> p k t", p=128)
            for k in ks:
                a2 = wp.tile([128, 512], BF, tag="arsb", bufs=3, name="ar2sb")
                nc.sync.dma_start(out=a2, in_=ar2v[:, k, :])
                xm = wp.tile([128, 512], BF, tag="xm2", bufs=2, name="xm2")
                nc.sync.dma_start(out=xm, in_=xmid_v[:, k, c0:c0 + 512])
                o_f = wp.tile([128, 512], F32, tag="o_f", bufs=2, name="o_f")
                nc.vector.scalar_tensor_tensor(
                    out=o_f, in0=a2, scalar=b2_sb[:, k:k + 1], in1=xm,
                    op0=Alu.add, op1=Alu.add)
                nc.sync.dma_start(
                    out=outT_v[k * 128:(k + 1) * 128, c0:c0 + 512], in_=o_f)

        u_box = {}
        w2p_box = {}

        # ---------- attention for one chunk (head-interleaved) ----------
        def att_block(ci, res1_head=1):
            nb = 4 * (ci + 1)
            ngrp = nb // 2
            qT = qT_tiles.pop(ci)
            dn8 = wp.tile([LH, 512], F32, tag="dn8", bufs=2, name="dn8")
            oT = wp.tile([128, KHE, 512], BF, tag="oT", bufs=1, name="oT")
            ou_map = {}
            ex_map = {}
            po_map = {}
            pending = []
            pending_hi = []
            state = {"i": 0, "hi": 0, "per": 0, "reserve": 0}

            def inject(ignore_reserve=False):
                lim = len(pending) - (0 if ignore_reserve else
                                      state["reserve"])
                for _ in range(state["per"]):
                    if state["hi"] < len(pending_hi):
                        pending_hi[state["hi"]]()
                        state["hi"] += 1
                    elif state["i"] < lim:
                        pending[state["i"]]()
                        state["i"] += 1

            def repace(h):
                # spread remaining thunks over remaining pair slots,
                # keeping `reserve` of them for the normalization window
                slots = (LH - 1 - h) * ngrp + ngrp
                rem = (len(pending) - state["i"] - state["reserve"]
                       + len(pending_hi) - state["hi"])
                state["per"] = max(1, -(-rem // max(slots, 1)))

            def emit_S(h, g):
                hp, hi = h // 2, h % 2
                e0 = hi * 64
                grp = psc.tile([128, 2, 512], F32, tag="ps_sc", bufs=2,
                               name="ps_sc")
                for j in range(2):
                    sb = 2 * g + j
                    mm(out=grp[:, j, :],
                       lhsT=kT[e0:e0 + 64, hp, sb * 128:(sb + 1) * 128],
                       rhs=qT[e0:e0 + 64, hp, :],
                       start=True, stop=True)
                exg = wp.tile([128, 2, 512], BF, tag="ex", bufs=10, name="ex")
                ex_map[(h, g)] = exg
                base = 4 * ci
                cuts = [max(0, (2 * g + j - base)) * 128
                        if 2 * g + j >= base else 0 for j in range(2)]
                if cuts == [0, 0]:
                    nc.scalar.activation(out=exg, in_=grp, func=Act.Exp,
                                         scale=float(HS) ** -0.5)
                else:
                    # causal: columns below the diagonal offset are dead --
                    # exp/mask/AV only touch the live range
                    for j in range(2):
                        cut = cuts[j]
                        nc.scalar.activation(out=exg[:, j, cut:512],
                                             in_=grp[:, j, cut:512],
                                             func=Act.Exp,
                                             scale=float(HS) ** -0.5)
                for j in range(2):
                    sb = 2 * g + j
                    if sb >= base:
                        cut = cuts[j]
                        nc.vector.tensor_mul(
                            out=exg[:, j, cut:cut + 128],
                            in0=exg[:, j, cut:cut + 128], in1=mask128)

            def emit_AV(h, g):
                if g == 0:
                    po_map[h] = poa.tile([65, 512], F32, tag="po", bufs=2,
                                         name="po")
                po = po_map[h]
                exg = ex_map.pop((h, g))
                base = 4 * ci
                for j in range(2):
                    sb = 2 * g + j
                    cut = (max(0, (sb - base)) * 128 if sb >= base else 0)
                    mm(out=po[:, cut:512],
                       lhsT=vS[:, sb, h * 65:h * 65 + 65],
                       rhs=exg[:, j, cut:512],
                       start=(sb == 0), stop=(sb == nb - 1))
                if g == ngrp - 1:
                    ou = wp.tile([64, 512], BF, tag="ou", bufs=9, name="ou")
                    ou_map[h] = ou
                    nc.vector.tensor_copy(out=ou, in_=po[0:64, :])
                    dnr = wp.tile([1, 512], F32, tag="dnr", bufs=2,
                                  name="dnr")
                    nc.vector.tensor_copy(out=dnr, in_=po[64:65, :])
                    nc.sync.dma_start(out=dn8[h:h + 1, :], in_=dnr)

            for h in range(LH):
                for g in range(ngrp):
                    emit_S(h, g)
                    if h > 0:
                        emit_AV(h - 1, g)
                    inject()
                if h == res1_head and ci >= 1:
                    res1_ln2(ci - 1)
                    pending.extend(fc1_thunks(ci - 1))
                    state["reserve"] = 10
                    repace(h)
                if h == 1 and ci == 0:
                    pending_hi.extend(ln1_thunks(1))
                    repace(h)
            for g in range(ngrp):
                emit_AV(LH - 1, g)
                inject()

            # normalization: o /= rowsum(exp); remaining thunks keep PE busy
            rec8 = wp.tile([LH, 512], F32, tag="rec8", bufs=1, name="rec8")
            nc.vector.reciprocal(out=rec8, in_=dn8)
            rb8 = wp.tile([LH, 512], BF, tag="rb8", bufs=1, name="rb8")
            nc.vector.tensor_copy(out=rb8, in_=rec8)
            for h in range(LH):
                rbt = wp.tile([1, 512], BF, tag="rbt", bufs=2, name="rbt")
                nc.sync.dma_start(out=rbt, in_=rb8[h:h + 1, :])
                bc = wp.tile([64, 512], BF, tag="bc", bufs=2, name="bc")
                nc.gpsimd.partition_broadcast(bc, rbt)
                nc.vector.tensor_mul(
                    out=oT[(h % 2) * 64:(h % 2) * 64 + 64, h // 2, :],
                    in0=ou_map[h], in1=bc)
                state["per"] = 2
                inject(ignore_reserve=True)
            state["per"] = len(pending) + len(pending_hi)
            inject(ignore_reserve=True)
            return oT

        # ---------- main schedule ----------
        ln1_load(0)
        load_weights()
        ln1_compute(0)
        for ci in range(NCH):
            qkv(ci)
            if ci + 1 < NCH:
                ln1_load(ci + 1)
            oT = att_block(ci, res1_head=(2 if ci == 1 else 1))
            if 1 <= ci < NCH - 1:
                ln1_compute(ci + 1)
            last = ci == NCH - 1
            wo_ar1(ci, oT, split=last)
            if ci >= 1:
                fc2_prefetch(ci - 1)
                fc2_ar2(ci - 1)
            if ci >= 2:
                res2(ci - 2)
        # ---------- tail ----------
        res1_ln2(NCH - 1)
        fc2_prefetch(NCH - 1)
        for th in fc1_thunks(NCH - 1):
            th()
        res2(NCH - 2)
        fc2_ar2(NCH - 1, split=True)
        res2(NCH - 1, ks=range(0, 4))
        res2(NCH - 1, ks=range(4, KD))

    attp.release()
    wres.release()
    consts.release()
    dram.release()


def _build():
    nc = bacc.Bacc("TRN2", target_bir_lowering=False, debug=False,
                   num_devices=NCORES)

    tensors = {}
    tensors["xbT"] = nc.dram_tensor("xbT", [NCH, 128, KD, 512], BF,
                                    kind="ExternalInput").ap()
    for name, shape, dt in (
        ("wq", [128, KD, LHE], BF), ("wk", [128, KD, LHE], BF),
        ("wv", [128, KD, LHE], BF),
        ("wo", [128, KHE, D], BF), ("w1", [KFF, 128, KD, 128], BF),
        ("w2", [KD, 128, KFF, 128], BF),
        ("b1l", [LFF], F32), ("bo", [D], F32), ("b2", [D], F32),
    ):
        tensors[name] = nc.dram_tensor(name, shape, dt,
                                       kind="ExternalInput").ap()
    tensors["outT"] = nc.dram_tensor("out", [D, T], F32,
                                     kind="ExternalOutput").ap()

    with tile.TileContext(nc, num_cores=NCORES) as tc:
        _emit(nc, tc, tensors)

    nc.compile()
    return nc


_NC_CACHE = None


def _get_nc():
    global _NC_CACHE
    if _NC_CACHE is None:
        _NC_CACHE = _build()
    return _NC_CACHE


def _shard_inputs(x, Wq, Wk, Wv, Wo, bo, W1, b1, W2, b2, g1, be1, g2, be2):
    """Build the 8 per-core input maps."""
    bf = lambda a: np.ascontiguousarray(a).astype(BF16NP)
    f32 = lambda a: np.ascontiguousarray(a, dtype=np.float32)

    in_maps = []
    for c in range(NCORES):
        b, half = divmod(c, TP)
        heads = slice(half * LH, (half + 1) * LH)
        ffs = slice(half * LFF, (half + 1) * LFF)
        hes = slice(half * LHE, (half + 1) * LHE)
        g1a = np.asarray(g1, dtype=np.float64)
        be1a = np.asarray(be1, dtype=np.float64)
        g2a = np.asarray(g2, dtype=np.float64)
        be2a = np.asarray(be2, dtype=np.float64)
        wq_l = np.concatenate(list(np.asarray(Wq)[heads]), axis=1)
        wk_l = np.concatenate(list(np.asarray(Wk)[heads]), axis=1)
        wv_l = np.concatenate(list(np.asarray(Wv)[heads]), axis=1)
        # fold LN1 gamma into Wq/Wk/Wv; q keeps a bias, k's bias cancels in
        # softmax, v's bias folds into bo (summed over ALL heads, not just
        # this core's, since both TP halves add bo after the AllReduce)
        # q bias be1@Wq is exactly zero for this model (be1 == 0)
        assert not np.any(be1a), "nonzero be1 needs a q-bias add"
        wq_l = wq_l * g1a[:, None]
        wk_l = wk_l * g1a[:, None]
        wv_all = np.concatenate(list(np.asarray(Wv)), axis=1)
        bo_f = (np.asarray(bo, dtype=np.float64)
                + (be1a @ wv_all) @ np.asarray(Wo, dtype=np.float64))
        wv_l = wv_l * g1a[:, None]
        # fold LN2 gamma/beta into W1/b1
        w1_loc = np.asarray(W1)[:, ffs]
        b1_f = np.asarray(b1, dtype=np.float64)[ffs] + be2a @ w1_loc
        w1_loc = w1_loc * g2a[:, None]
        # pre-shuffle so every on-chip tile/piece is DRAM-contiguous
        shuf_kp = lambda w, wid: np.ascontiguousarray(
            w.reshape(KD, 128, wid).transpose(1, 0, 2))      # [128, KD, wid]
        w1_l = w1_loc.reshape(KD, 128, KFF, 128)
        w1_l = np.ascontiguousarray(w1_l.transpose(2, 1, 0, 3))
        w2_l = np.asarray(W2)[ffs, :].reshape(KFF, 128, KD, 128)
        w2_l = np.ascontiguousarray(w2_l.transpose(2, 1, 0, 3))
        xb_l = np.asarray(x)[b].T.reshape(KD, 128, NCH, 512)
        xb_l = np.ascontiguousarray(xb_l.transpose(2, 1, 0, 3))
        wo_l = np.asarray(Wo)[hes, :].reshape(KHE, 128, D)
        wo_l = np.ascontiguousarray(wo_l.transpose(1, 0, 2))
        in_maps.append({
            "xbT": bf(xb_l),
            "wq": bf(shuf_kp(wq_l, LHE)), "wk": bf(shuf_kp(wk_l, LHE)),
            "wv": bf(shuf_kp(wv_l, LHE)),
            "wo": bf(wo_l),
            "w1": bf(w1_l), "w2": bf(w2_l),
            "b1l": f32(b1_f),
            "bo": f32(bo_f), "b2": f32(b2),
        })
    return in_maps


def kernel(x, Wq, Wk, Wv, Wo, bo, W1, b1, W2, b2, g1, be1, g2, be2,
           _trace=False):
    nc = _get_nc()
    in_maps = _shard_inputs(x, Wq, Wk, Wv, Wo, bo, W1, b1, W2, b2,
                            g1, be1, g2, be2)
    res = run_bass_kernel_spmd(nc, in_maps, list(range(NCORES)),
                               trace=_trace)
    out = np.empty((B, T, D), dtype=np.float32)
    for b in range(B):
        out[b] = res.results[TP * b]["out"].T
    if _trace:
        kernel.last_exec_time_ns = res.exec_time_ns
        kernel.last_results = res
    return out
